# revision 9
# baseline (speedup 1.0000x reference)
"""Trainium2 Bass kernel for HQNN-Quanv (B=1024, 1x28x28, K=2) — v2.

Math: circuit weights are 0 so the quantum circuit is 3 CNOTs. Per 2x2 patch
with c_k = cos(pi*p_k): <Z0>=c0, <Z1>=c1, <Z2>=c0*c2, <Z3>=c0*c2*c3, then
y = feat @ W.T + b. Device computes s = sin(pi(x-0.5)) = -cos(pi x); sign
flips are folded into host-prepared weights.

v2 structure (per core, batch/8 = 128 images):
  - ONE input DRAM tensor [128, 2622] fp16 = [wt 190 | xlin 896 | xb 768 | xc 768].
  - DMA1 (wt+xlin) is HOISTED into block 0 before the Tile preamble barrier,
    so HWDGE descriptor-gen overlaps the engine register setup (~1us earlier
    data arrival). DMA2 (xb) / DMA3 (xc) follow on the SP ring in body.
  - ACT: sin(sl) whole, then sb/sc in two 3-chunk groups interleaved
    (sl -> sb012 -> sc012 -> sb345 -> sc345), pinned order.
  - DVE: e2_g = sl_g*sb_g, e3_g = e2_g*sc_g streamed per group.
  - PE: 7 lin matmuls after sl; per group 3 e2 + 3 e3 matmuls streamed; all
    accumulate into one PSUM tile [10, 128].
  - DVE copy PSUM->SBUF, SP DMA out.
  - lean kernel tail (single barrier) + multi-wait splitting for walrus.
"""

import sys

if "/opt/trn_rl_repo" not in sys.path:
    sys.path.insert(0, "/opt/trn_rl_repo")

import numpy as np

B = 1024
NCORES = 8
BC = B // NCORES  # 128 images per core
H = 28
F = 27
NLIN = 7  # ceil(784/128) chunks for the linear (c) term
NE = 6  # ceil(756/128) chunks for the E2/E3 terms
FREE_LIN = NLIN * 128  # 896
FREE_E = NE * 128  # 768
WCOLS = (NLIN + 2 * NE) * 10  # 190
BIAS_SLOT = 784  # first pad slot in the linear chunk space
XIN_COLS = WCOLS + FREE_LIN + 2 * FREE_E  # 2622
GRP = 3 * 128  # 384-col sb/sc groups

_cached_nc = None


def _lean_drain_and_barrier(self, tick_clock, wait_clock):
    """Tail for a one-shot NEFF: final-value waits + one barrier + sem
    cleanup, skipping the trailing all-engine barrier."""
    from concourse.vector_clock import ScopedClock

    drain_inst = self.nc.sync.drain()
    wait_clock.add_sem_waits(
        drain_inst.ins, ScopedClock({None: tick_clock.global_clock})
    )
    self.nc.all_engine_barrier()
    popped = self.nc._tile_sem_poison_stack.pop()
    assert popped is self._sem_poison
    self.nc.clear_and_free_semaphores(list(self.sems.allocated().values()))


def build_nc():
    import concourse.bass as bass
    import concourse.tile as tile
    import concourse.mybir as mybir
    from concourse.bass import _add_dep_helper

    nc = bass.Bass("TRN2", target_bir_lowering=False, debug=False)
    f16 = mybir.dt.float16
    f32 = mybir.dt.float32
    i8 = mybir.dt.int8
    xd = nc.dram_tensor("xd", [128, FREE_LIN + 2 * FREE_E], i8, kind="ExternalInput")
    wd = nc.dram_tensor("wd", [128, WCOLS], f16, kind="ExternalInput")
    y = nc.dram_tensor("y", [BC, 10], f32, kind="ExternalOutput")

    tc = tile.TileContext(nc)
    tc._drain_and_barrier = _lean_drain_and_barrier.__get__(tc)
    hoist = None
    with tc:
        with (
            tc.tile_pool(name="p", bufs=1) as pool,
            tc.tile_pool(name="ps", bufs=1, space="PSUM") as pp,
        ):
            # int8 x stream (halves the pair-contended HBM bytes); the SIN
            # dequantizes for free via scale = pi/254
            xlt = pool.tile([128, FREE_LIN], i8)
            hoist = nc.sync.dma_start(xlt[:], xd.ap()[:, 0:FREE_LIN])
            xb = pool.tile([128, FREE_E], i8)
            nc.sync.dma_start(xb[:], xd.ap()[:, FREE_LIN : FREE_LIN + FREE_E])
            xc = pool.tile([128, FREE_E], i8)
            nc.sync.dma_start(
                xc[:], xd.ap()[:, FREE_LIN + FREE_E : FREE_LIN + 2 * FREE_E]
            )
            wl = pool.tile([128, WCOLS], f16)
            nc.sync.dma_start(wl[:], wd.ap())
            wt = wl[:]
            xlin = xlt[:]

            sin = mybir.ActivationFunctionType.Sin
            pi = float(np.pi)
            sl = pool.tile([128, FREE_LIN], f16)
            sb = pool.tile([128, FREE_E], f16)
            sc = pool.tile([128, FREE_E], f16)
            a_sl = nc.scalar.activation(sl[:], xlin[:], sin, bias=0.0, scale=pi / 254.0)
            # sb gates on the single xb-DMA semaphore: one full-width SIN.
            a_b = nc.scalar.activation(sb[:], xb[:], sin, bias=0.0, scale=pi / 254.0)
            _add_dep_helper(a_b.ins, a_sl.ins, False, "pin ACT order")
            # sc split in two groups so e3 muls / PE stream behind each half.
            prev = a_b
            for g in range(2):
                o = g * GRP
                a_c = nc.scalar.activation(
                    sc[:, o : o + GRP], xc[:, o : o + GRP], sin, bias=0.0, scale=pi / 254.0
                )
                _add_dep_helper(a_c.ins, prev.ins, False, "pin ACT order")
                prev = a_c

            e2 = pool.tile([128, FREE_E], f16)
            e3 = pool.tile([128, FREE_E], f16)
            nc.vector.tensor_mul(e2[:], sl[:, 0:FREE_E], sb[:])
            for g in range(2):
                o = g * GRP
                nc.vector.tensor_mul(
                    e3[:, o : o + GRP], e2[:, o : o + GRP], sc[:, o : o + GRP]
                )

            # Dummy matmul reading only wt: absorbs the wl-DMA semaphore wait
            # on the PE so the first real matmul carries a single wait.
            scratch = pp.tile([10, 10], f32)
            nc.tensor.matmul(scratch[:], wt[:, 0:10], wt[:, 0:10])

            yp = pp.tile([BC, 10], f32)
            nmm = NLIN + 2 * NE
            i = 0
            for t in range(NLIN):
                nc.tensor.matmul(
                    yp[:],
                    sl[:, t * 128 : (t + 1) * 128],
                    wt[:, t * 10 : (t + 1) * 10],
                    start=(i == 0),
                    stop=(i == nmm - 1),
                )
                i += 1
            for t in range(NE):
                nc.tensor.matmul(
                    yp[:],
                    e2[:, t * 128 : (t + 1) * 128],
                    wt[:, NLIN * 10 + t * 10 : NLIN * 10 + (t + 1) * 10],
                    start=(i == 0),
                    stop=(i == nmm - 1),
                )
                i += 1
            wofs = (NLIN + NE) * 10
            for g in range(2):
                for t in range(g * 3, g * 3 + 3):
                    nc.tensor.matmul(
                        yp[:],
                        e3[:, t * 128 : (t + 1) * 128],
                        wt[:, wofs + t * 10 : wofs + (t + 1) * 10],
                        start=(i == 0),
                        stop=(i == nmm - 1),
                    )
                    i += 1

            ys = pool.tile([BC, 10], f32)
            nc.vector.tensor_copy(ys[:], yp[:])
            nc.sync.dma_start(y.ap(), ys[:])

    _hoist_to_block0(nc, hoist.ins)
    _trim_preamble(nc)
    _drop_out_dma_tail_wait(nc)
    _split_multi_waits(nc)
    return nc


def _drop_out_dma_tail_wait(nc):
    """Remove the kernel-tail drain's wait on the output DMA's completion
    semaphore. The tail's gpsimd dma_reset DRAINS in-flight DMAs before
    resetting, so the store still completes before the NEFF finishes — but
    the ~0.9us completion-sem propagation round-trip leaves the critical
    path, and the barrier/cleanup overlap the transfer."""
    out_sem = None
    blocks = list(nc.m.functions[0].blocks)
    for blk in blocks:
        for i in blk.instructions:
            if type(i).__name__ == "InstDMACopy":
                for o in i.outs:
                    if getattr(o, "memref", "") == "y":
                        for u in i.sync_info.on_update or []:
                            out_sem = u.id
    assert out_sem is not None
    tail = blocks[-1]
    for i in tail.instructions:
        si = i.sync_info
        if si and si.on_wait:
            kept = [w for w in si.on_wait if w.id != out_sem]
            if len(kept) != len(si.on_wait):
                si.on_wait = kept


def _trim_preamble(nc):
    """Drop dead Tile-preamble work so the block-0 barrier releases earlier:
    bounds-check register moves (only dynamic-AP DMAs read them; all DMAs
    here are static) and memsets of const tiles no instruction reads."""
    import re

    blk0 = list(nc.m.functions[0].blocks)[0]
    keep = []
    for i in list(blk0.instructions):
        tn = type(i).__name__
        if tn == "InstRegisterMove":
            m = re.search(r"regref='([^']+)'", str(i.outs[0]))
            if m and "bcreg" in m.group(1):
                continue
        if tn == "InstMemset":
            mr = str(getattr(i.outs[0], "memref", ""))
            if mr.startswith("const-") and mr != "const-float32-0.0":
                continue
        keep.append(i)
    blk0.instructions[:] = keep


def _hoist_to_block0(nc, dma_inst):
    """Move the first input DMA to the top of block 0 (before the Tile
    preamble register moves + barrier) so HWDGE descriptor generation and the
    transfer overlap the engine preamble. The DMA has a static physical AP
    (no bc-register use) and no sem waits; its completion sem still gates
    downstream consumers."""
    blocks = list(nc.m.functions[0].blocks)
    src = None
    for blk in blocks:
        insts = list(blk.instructions)
        if dma_inst in insts:
            src = blk
            break
    assert src is not None, "hoist target not found"
    src.instructions.remove(dma_inst)
    blk0 = blocks[0]
    first = list(blk0.instructions)
    # insert after the leading dummy call, before the register moves
    pos = 1 if first and type(first[0]).__name__ == "InstCall" else 0
    blk0.instructions.insert(pos, dma_inst)


def _split_multi_waits(nc):
    """Walrus allows only one sync-wait per instruction; split any multi-wait
    instruction into preceding single-wait NoOps on the same engine."""
    import concourse.mybir as mybir

    ctr = 0
    for blk in nc.m.functions[0].blocks:
        new_insts = []
        changed = False
        for inst in blk.instructions:
            si = inst.sync_info
            if si is not None and si.on_wait and len(si.on_wait) > 1:
                waits = list(si.on_wait)
                for w in waits[:-1]:
                    nop = mybir.InstNoOp(name=f"I-splitw-{ctr}", ins=[], outs=[])
                    ctr += 1
                    nop.engine = inst.engine
                    nop.sync_info = mybir.SyncInfo(on_wait=[w], on_update=[])
                    nc.register_instruction(nop, overwrite=True)
                    new_insts.append(nop)
                si.on_wait = waits[-1:]
                changed = True
            new_insts.append(inst)
        if changed:
            blk.instructions[:] = new_insts


def prep_x_core(xs):
    """xs: (BC, 28, 28) float32 -> (xlin, xb, xc) fp16 slot layouts."""
    u = xs.reshape(BC, H * H).astype(np.float64) - 0.5
    q = np.clip(np.rint(u * 254.0), -127, 127).astype(np.int8)
    ut = q.T  # (784, BC) int8

    ulin = np.zeros((FREE_LIN, BC), np.int8)
    ulin[: H * H] = ut
    ulin[BIAS_SLOT] = 127  # bias slot: sin(pi*127/254) = 1
    xlin = ulin.reshape(NLIN, 128, BC).transpose(1, 0, 2).reshape(128, FREE_LIN)

    ub = np.zeros((FREE_E, BC), np.int8)
    ub[:756] = ut[28:784]
    xbm = ub.reshape(NE, 128, BC).transpose(1, 0, 2).reshape(128, FREE_E)

    uc = np.zeros((FREE_E, BC), np.int8)
    uc[:755] = ut[29:784]
    xcm = uc.reshape(NE, 128, BC).transpose(1, 0, 2).reshape(128, FREE_E)

    return xlin, xbm, xcm


def prep_w(W, b):
    """W: (10, 2916), b: (10,) -> wt (128, WCOLS) fp16.

    Device computes s = -cos(pi*x); sign folds: lin -> -A, E2 -> +W2,
    E3 -> -W3 (since e3_dev = -c0*c2*c3)."""
    W = W.astype(np.float32)
    W0 = W[:, 0:729].reshape(10, F, F)
    W1 = W[:, 729:1458].reshape(10, F, F)
    W2 = W[:, 1458:2187].reshape(10, F, F)
    W3 = W[:, 2187:2916].reshape(10, F, F)

    A = np.zeros((10, H, H), np.float32)
    A[:, :F, :F] += W0
    A[:, :F, 1:H] += W1

    wlin = np.zeros((10, FREE_LIN), np.float32)
    wlin[:, : H * H] = -A.reshape(10, H * H)
    wlin[:, BIAS_SLOT] = b
    wlin_p = wlin.reshape(10, NLIN, 128).transpose(2, 1, 0).reshape(128, NLIN * 10)

    w2s = np.zeros((10, FREE_E), np.float32)
    w2s[:, :756].reshape(10, F, H)[:, :, :F] = W2
    w2_p = w2s.reshape(10, NE, 128).transpose(2, 1, 0).reshape(128, NE * 10)

    w3s = np.zeros((10, FREE_E), np.float32)
    w3s[:, :756].reshape(10, F, H)[:, :, :F] = -W3
    w3_p = w3s.reshape(10, NE, 128).transpose(2, 1, 0).reshape(128, NE * 10)

    return np.concatenate([wlin_p, w2_p, w3_p], axis=1).astype(np.float16)


def _get_nc():
    global _cached_nc
    if _cached_nc is None:
        _cached_nc = build_nc()
    return _cached_nc


def _make_in_maps(inputs):
    x = np.asarray(inputs["x"], np.float32)
    W = np.asarray(inputs["W"], np.float32)
    b = np.asarray(inputs["b"], np.float32)
    wt = prep_w(W, b)
    in_maps = []
    for k in range(NCORES):
        xs = x[k * BC : (k + 1) * BC, 0]
        xlin, xbm, xcm = prep_x_core(xs)
        in_maps.append(
            {"xd": np.concatenate([xlin, xbm, xcm], axis=1), "wd": wt}
        )
    return in_maps


def run(inputs, trace=False, **kwargs):
    from concourse.bass_utils import run_bass_kernel_spmd

    nc = _get_nc()
    in_maps = _make_in_maps(inputs)
    res = run_bass_kernel_spmd(
        nc, in_maps, core_ids=list(range(NCORES)), trace=trace, **kwargs
    )
    out = np.concatenate([r["y"] for r in res.results], axis=0)
    return out, res


def kernel(**inputs) -> np.ndarray:
    out, _ = run(inputs, trace=False)
    return out
